# revision 1
# baseline (speedup 1.0000x reference)
"""DigitCaps dynamic-routing kernel for Trainium2 (8 NeuronCores, Bass/Tile).

Problem: B=256, IN_CAPS=3200, IN_DIM=8, OUT_CAPS=8, OUT_DIM=16, 3 routing
iterations.  Data-parallel over batch: 32 batches per core.

v2 design (vs. the 775us baseline):
  - 3-stage software-pipelined emission per routing block: creation(t) on
    PE overlaps a-pass+softmax(t-2) and s-pass(t-3), so no engine waits on
    the per-tile cross-engine chain.
  - Creation PSUM->SBUF copies split DVE/Act; Pool (no PSUM port) takes
    softmax reduce + c-mult + vblk build.
  - Squash runs in [jm-partition, batch] layout end to end: the sum over m
    and the j->jm broadcast are two tiny PE matmuls with mask operands, so
    the XBAR transposes and [batch-partition] round trip are gone.
  - Softmax exp reads the a-pass PSUM directly (it2); logits add (it3) on
    DVE.  s accumulates over t in PSUM with start/stop flags (no memset).
  - PSUM packed exactly into 8 banks: s-accum 4, creation ping-pong 2,
    a-pass/squash shared pool 2.
"""

import sys

if "/opt/trn_rl_repo" not in sys.path:
    sys.path.insert(0, "/opt/trn_rl_repo")

import ml_dtypes
import numpy as np

import bass_rust
import concourse.bass as bass
import concourse.mybir as mybir
import concourse.tile as tile
from concourse._compat import with_exitstack
from concourse.bass_utils import run_bass_kernel_spmd
from concourse.vector_clock import ScopedClock

# ---------------------------------------------------------------------------
# Walrus on this toolchain rejects multi-wait CTRL instructions;
# TileContext's tail drain aggregates one wait per outstanding semaphore.
# Split the waits across consecutive SP drains.
_TILE_PATCHED = False


def _drain_and_barrier_split(self, tick_clock, wait_clock):
    drain_inst = self.nc.sync.drain()
    wait_clock.add_sem_waits(
        drain_inst.ins, ScopedClock({None: tick_clock.global_clock})
    )
    mi = drain_inst.ins
    waits = list(mi.sync_info.on_wait) if mi.sync_info else []
    if len(waits) > 1:
        si = mi.sync_info
        si.on_wait = waits[:1]
        mi.sync_info = si
        for i in range(1, len(waits)):
            extra = self.nc.sync.drain().ins
            extra.sync_info = bass_rust.SyncInfo(
                on_wait=waits[i : i + 1], on_update=[]
            )
    self.nc.all_engine_barrier()
    assert self.sems is not None
    popped = self.nc._tile_sem_poison_stack.pop()
    assert popped is self._sem_poison
    self.nc.clear_and_free_semaphores(list(self.sems.allocated().values()))
    self.nc.all_engine_barrier()


def _patch_tile():
    global _TILE_PATCHED
    if not _TILE_PATCHED:
        tile.TileContext._drain_and_barrier = _drain_and_barrier_split
        _TILE_PATCHED = True


_SW_COUNT = [0]


def _split_waits(nc):
    """This walrus build allows one sync wait per instruction: hoist extra
    waits onto same-engine NoOp carriers placed just before."""
    for f in nc.m.functions:
        for blk in f.blocks:
            insts = blk.instructions
            if not any(
                inst.sync_info and len(inst.sync_info.on_wait) > 1
                for inst in insts
            ):
                continue
            new = []
            for inst in insts:
                si = inst.sync_info
                waits = list(si.on_wait) if si else []
                if len(waits) > 1:
                    for w in waits[:-1]:
                        _SW_COUNT[0] += 1
                        car = mybir.InstNoOp(
                            name=f"I-sw{_SW_COUNT[0]}", engine=inst.engine
                        )
                        car.sync_info = bass_rust.SyncInfo(
                            on_wait=[w], on_update=[]
                        )
                        new.append(car)
                    si.on_wait = waits[-1:]
                    inst.sync_info = si
                new.append(inst)
            insts[:] = new


# ---------------------------------------------------------------------------
B, I, N, J, M = 256, 3200, 8, 8, 16
JM = J * M  # 128
N_CORES = 8
B_C = B // N_CORES  # 32
T = I // 128  # 25 i-tiles

IP = 16  # i's packed per K-chunk (K = IP*N = 128, uniform row group)
H = I // IP  # 200
CH_T = 128 // IP  # 8 creation chunks per 128-i tile

F32 = mybir.dt.float32
BF16 = mybir.dt.bfloat16
FP8 = mybir.dt.bfloat16  # fp8 inputs cost 3.6% rel err (routing amplifies) - stay bf16
W_SCALE = 1.0
AF = mybir.ActivationFunctionType
ALU = mybir.AluOpType


XB_ENG = [lambda nc: nc.sync]
TR_ENG = [lambda nc, ta: nc.scalar]


def _bc(t, dims):
    """Broadcast AP helper: dims is a list of either an existing ap pair or
    [0, n] broadcast entries."""
    return bass.AP(t.tensor, t[:].offset, dims)


def _squash_jm(nc, sqp, psX, src_ps, mask32, maskT32, nb, tag, pre_scale=None):
    """src_ps [JM, nb] f32 (PSUM) -> (s_sb [JM, nb] f32 SBUF,
    scale_ps [JM, nb] f32 PSUM) all in jm-partition layout.
    squash scale per capsule j: sq = sum_m s^2, scale = sq/((1+sq)*sqrt(sq))."""
    s_sb = sqp.tile([JM, nb], F32, tag=f"ssb{tag}", bufs=1 if tag == "a" else 2)
    if pre_scale is None:
        nc.scalar.activation(s_sb[:], src_ps[:], AF.Copy)
    else:
        nc.vector.tensor_scalar_mul(s_sb[:], src_ps[:], pre_scale)
    s2 = sqp.tile([JM, nb], F32, tag=f"s2{tag}", bufs=1 if tag == "a" else 2)
    nc.vector.tensor_tensor(s2[:], s_sb[:], s_sb[:], ALU.mult)
    # sq[j, b] = sum over the 16 m-rows of j:  mask32 [jm(K), j] stationary
    sq_t = psX.tile([J, B_C], F32, tag="x", padded_shape=[J, 512])
    sq_ps = sq_t[:, :nb]
    nc.tensor.matmul(sq_ps, mask32[:], s2[:], start=True, stop=True)
    rt = sqp.tile([J, nb], F32, tag=f"rt{tag}")
    nc.scalar.activation(rt[:], sq_ps, AF.Sqrt)
    den = sqp.tile([J, nb], F32, tag=f"den{tag}")
    nc.vector.tensor_scalar_add(den[:], sq_ps, 1.0)
    nc.vector.tensor_tensor(den[:], den[:], rt[:], ALU.mult)
    rden = sqp.tile([J, nb], F32, tag=f"rd{tag}")
    nc.vector.reciprocal(rden[:], den[:])
    scale = sqp.tile([J, nb], F32, tag=f"sc{tag}")
    nc.vector.tensor_tensor(scale[:], sq_ps, rden[:], ALU.mult)
    # broadcast j -> jm rows:  maskT32 [j(K), jm] stationary
    scale_t = psX.tile([JM, B_C], F32, tag="x", padded_shape=[JM, 512])
    scale_ps = scale_t[:, :nb]
    nc.tensor.matmul(scale_ps, maskT32[:], scale[:], start=True, stop=True)
    return s_sb, scale_ps


@with_exitstack
def build_kernel(ctx, tc, outs, ins, b_c=B_C, half=16, b_blk=2, reps=1, stage=3):
    nc = tc.nc
    (v_out,) = outs
    (wcr_d, xblk_d, xt_d, mask_d, ident_d) = ins
    n_half = b_c // half

    const = ctx.enter_context(tc.tile_pool(name="const", bufs=1))
    res = ctx.enter_context(tc.tile_pool(name="res", bufs=1))
    utp = ctx.enter_context(tc.tile_pool(name="ut", bufs=4))
    xs = ctx.enter_context(tc.tile_pool(name="xs", bufs=4))
    sm = ctx.enter_context(tc.tile_pool(name="sm", bufs=2))
    sqp = ctx.enter_context(tc.tile_pool(name="sq", bufs=2))
    vbp = ctx.enter_context(tc.tile_pool(name="vb", bufs=2))
    ps2 = ctx.enter_context(tc.tile_pool(name="ps2", bufs=3, space="PSUM"))
    psS = ctx.enter_context(tc.tile_pool(name="psS", bufs=1, space="PSUM"))
    psX = ctx.enter_context(tc.tile_pool(name="psX", bufs=1, space="PSUM"))

    # Resident constants.  wcr split into 4 chunks so iteration-1 matmuls
    # can start before the full 6.5MB lands.
    wcr = const.tile([128, H, JM], FP8)
    for q in range(4):
        hq = H // 4
        nc.sync.dma_start(
            wcr[:, q * hq : (q + 1) * hq, :], wcr_d[:, q * hq : (q + 1) * hq, :]
        )
    xt = const.tile([128, H, b_c], FP8)
    nc.sync.dma_start(xt[:], xt_d[:])
    mask_rep = const.tile([JM, J], BF16)
    nc.sync.dma_start(mask_rep[:], mask_d[:])
    maskT = const.tile([J, JM], BF16)
    nc.sync.dma_start(maskT[:], mask_d[:].rearrange("a b -> b a"))
    mask32 = const.tile([JM, J], F32)
    nc.vector.tensor_copy(mask32[:], mask_rep[:])
    maskT32 = const.tile([J, JM], F32)
    nc.scalar.activation(maskT32[:], maskT[:], AF.Copy)
    ones8 = const.tile([J, 1], BF16)
    nc.vector.memset(ones8[:], 1.0)
    ident = const.tile([128, 128], F32)
    nc.sync.dma_start(ident[:], ident_d[:])

    for rep in range(reps):
        vblk = vbp.tile([JM, b_c, 2, J], BF16, tag="vblk")
        s_ps = psS.tile([J, half, JM], F32, tag="s_ps")

        # ---- iteration 1 (all batches): s1 = (1/8) sum_(i,n) W x ----------
        s1_ps = psX.tile([JM, b_c], F32, tag="x", padded_shape=[JM, 512])
        for h in range(H):
            nc.tensor.matmul(
                s1_ps[:], wcr[:, h, :], xt[:, h, :],
                start=(h == 0), stop=(h == H - 1),
            )
        s_sb1, scale1 = _squash_jm(
            nc, sqp, psX, s1_ps, mask32, maskT32, b_c, "a",
            pre_scale=1.0 / (J * W_SCALE)
        )
        v16 = sqp.tile([JM, b_c], BF16, tag="v16")
        nc.vector.tensor_tensor(v16[:], s_sb1[:], scale1, ALU.mult)
        # vblk[:, :, 0, :] = v16 (bcast over J) * mask_rep (bcast over b)
        nc.gpsimd.tensor_tensor(
            vblk[:, :, 0, :],
            _bc(v16, [v16[:].ap[0], v16[:].ap[1], [0, J]]),
            _bc(mask_rep, [mask_rep[:].ap[0], [0, b_c], mask_rep[:].ap[1]]),
            ALU.mult,
        )

        for hf in range(n_half):
            b0 = hf * half
            u_res_t = [
                res.tile([128, half, JM], BF16, tag=f"ur{t}", name=f"ur{t}")
                for t in range(T)
            ]
            for it in (2, 3):
                nslot = it - 1
                if stage >= 3:
                    nc.vector.memset(s_ps[:], 0.0)
                state = {}
                xbs = {}
                SKEW = 8
                for u in range(T + SKEW):
                    ta = u  # creation tile
                    tb = u - 2  # a-pass + softmax tile
                    tc_ = u - SKEW  # s-pass tile

                    # ---- xb prefetch (2 tiles ahead of creation) ---------
                    pf = [0, 1, 2] if u == 0 else ([u + 2] if u + 2 < T else [])
                    for tp in pf:
                        xbp = xs.tile(
                            [128, CH_T, half, IP], FP8, tag="xb", name="xbp"
                        )
                        XB_ENG[0](nc).dma_start(xbp[:], xblk_d[hf, tp])
                        xbs[tp] = xbp

                    # ---- stage A part 1: creation pairs 0,1 --------------
                    if ta < T:
                        xb = xbs.pop(ta)
                        u_t = utp.tile([JM, half, 128], BF16, tag="u_t")
                        u_tv = u_t[:].rearrange("p b (hh i) -> p hh b i", i=IP)
                        for pair in (0, 1):
                            cps = ps2.tile([JM, 2, half, IP], F32, tag="cps")
                            for k in (0, 1):
                                hh = pair * 2 + k
                                nc.tensor.matmul(
                                    cps[:, k, :, :],
                                    wcr[:, ta * CH_T + hh, :],
                                    xb[:, hh, :, :],
                                    start=True, stop=True,
                                )
                            if pair == 0:
                                nc.scalar.activation(
                                    u_tv[:, 0:2], cps[:], AF.Copy,
                                    scale=1.0 / W_SCALE,
                                )
                            else:
                                nc.vector.tensor_scalar_mul(
                                    u_tv[:, 2:4], cps[:], 1.0 / W_SCALE
                                )
                        state[ta] = [u_t, None]

                    # ---- stage B part 1: a-pass first 8 b's --------------
                    if 0 <= tb < T and stage >= 2:
                        u_tb = state[tb][0]
                        aps = psX.tile(
                            [128, half, nslot * J], F32, tag="x",
                            padded_shape=[128, half, 32],
                        )
                        for b in range(half // 2):
                            nc.tensor.matmul(
                                aps[:, b, :],
                                u_tb[:, b, :],
                                vblk[:, b0 + b, :nslot, :],
                                start=True, stop=True,
                            )

                    # ---- stage A part 2: creation pairs 2,3 --------------
                    if ta < T:
                        for pair in (2, 3):
                            cps = ps2.tile([JM, 2, half, IP], F32, tag="cps")
                            for k in (0, 1):
                                hh = pair * 2 + k
                                nc.tensor.matmul(
                                    cps[:, k, :, :],
                                    wcr[:, ta * CH_T + hh, :],
                                    xb[:, hh, :, :],
                                    start=True, stop=True,
                                )
                            if pair == 2:
                                nc.scalar.activation(
                                    u_tv[:, 4:6], cps[:], AF.Copy,
                                    scale=1.0 / W_SCALE,
                                )
                            else:
                                nc.vector.tensor_scalar_mul(
                                    u_tv[:, 6:8], cps[:], 1.0 / W_SCALE
                                )
                        if it == 2:
                            TR_ENG[0](nc, ta).dma_start_transpose(
                                u_res_t[ta][:], u_t[:, :, :]
                            )
                            # WAR guard: a DVE read of both the XBAR src and
                            # dst; it can only run once the transpose's data
                            # is fully read, and its engine-side completion
                            # sem (reliable on HW, unlike the DMA read sem)
                            # then carries the u_t slot-reuse dependency.
                            xg = sm.tile([1, 4], F32, tag="xg", bufs=3)
                            nc.vector.tensor_tensor(
                                xg[:], u_t[0:1, 0, 0:128:32],
                                u_res_t[ta][0:1, 0, 0:4], ALU.add,
                            )

                    # ---- stage B part 2: a-pass rest + softmax -----------
                    if 0 <= tb < T and stage >= 2:
                        for b in range(half // 2, half):
                            nc.tensor.matmul(
                                aps[:, b, :],
                                u_tb[:, b, :],
                                vblk[:, b0 + b, :nslot, :],
                                start=True, stop=True,
                            )
                        e = sm.tile([128, half, J], BF16, tag="e")
                        if it == 2:
                            nc.scalar.activation(e[:], aps[:], AF.Exp)
                        else:
                            # exp(a1+a2) = exp(a1)*exp(a2); avoids a two-PSUM
                            # operand add (IBVF027)
                            av = aps[:].rearrange(
                                "p b (s j) -> p b s j", j=J
                            )
                            e0 = sm.tile([128, half, J], BF16, tag="e0", bufs=1)
                            nc.scalar.activation(e0[:], av[:, :, 0, :], AF.Exp)
                            e1 = sm.tile([128, half, J], BF16, tag="e1", bufs=1)
                            nc.scalar.activation(e1[:], av[:, :, 1, :], AF.Exp)
                            nc.gpsimd.tensor_tensor(
                                e[:], e0[:], e1[:], ALU.mult
                            )
                        z = sm.tile([128, half], F32, tag="z")
                        nc.vector.tensor_reduce(
                            z[:], e[:], mybir.AxisListType.X, ALU.add
                        )
                        rz = sm.tile([128, half], F32, tag="rz")
                        nc.vector.reciprocal(rz[:], z[:])
                        c_t = sm.tile([128, half, J], BF16, tag="c_t", bufs=7)
                        nc.gpsimd.tensor_tensor(
                            c_t[:],
                            e[:],
                            _bc(rz, [rz[:].ap[0], rz[:].ap[1], [0, J]]),
                            ALU.mult,
                        )
                        state[tb][1] = c_t

                    # ---- stage C: s-pass ---------------------------------
                    if tc_ >= 0 and stage >= 3:
                        c_tc = state.pop(tc_)[1]
                        assert c_tc is not None
                        for b in range(half):
                            nc.tensor.matmul(
                                s_ps[:, b, :],
                                c_tc[:, b, :],
                                u_res_t[tc_][:, b, :],
                                start=False,
                                stop=False,
                                skip_group_check=True,
                            )

                # ---- block boundary: extract s, squash -------------------
                if stage < 3:
                    continue
                # msb = s_ps masked to the j-diagonal; split in two DVE ops
                # so the first extract matmuls start during the second.
                msb = sqp.tile([J, half, JM], BF16, tag="msb", bufs=1)
                hh2 = half // 2
                mTb = _bc(
                    maskT, [maskT[:].ap[0], [0, hh2], maskT[:].ap[1]]
                )
                nc.vector.tensor_tensor(
                    msb[:, :hh2, :], s_ps[:, :hh2, :], mTb, ALU.mult
                )
                s2_ps = psX.tile([JM, half], F32, tag="x", padded_shape=[JM, 512])
                for b in range(hh2):
                    nc.tensor.matmul(
                        s2_ps[:, b : b + 1], msb[:, b, :], ones8[:],
                        start=True, stop=True,
                    )
                nc.vector.tensor_tensor(
                    msb[:, hh2:, :], s_ps[:, hh2:, :], mTb, ALU.mult
                )
                for b in range(hh2, half):
                    nc.tensor.matmul(
                        s2_ps[:, b : b + 1], msb[:, b, :], ones8[:],
                        start=True, stop=True,
                    )
                s_sb, scale_ps = _squash_jm(
                    nc, sqp, psX, s2_ps, mask32, maskT32, half, "b"
                )
                if it == 2:
                    v16h = sqp.tile([JM, half], BF16, tag="v16h")
                    nc.vector.tensor_tensor(
                        v16h[:], s_sb[:], scale_ps, ALU.mult
                    )
                    nc.gpsimd.tensor_tensor(
                        vblk[:, b0 : b0 + half, 1, :],
                        _bc(v16h, [v16h[:].ap[0], v16h[:].ap[1], [0, J]]),
                        _bc(
                            mask_rep,
                            [mask_rep[:].ap[0], [0, half], mask_rep[:].ap[1]],
                        ),
                        ALU.mult,
                    )
                else:
                    v32 = sqp.tile([JM, half], F32, tag="v32")
                    nc.vector.tensor_tensor(
                        v32[:], s_sb[:], scale_ps, ALU.mult
                    )
                    vT_ps = psX.tile([half, JM], F32, tag="x", padded_shape=[half, 512])
                    nc.tensor.matmul(
                        vT_ps[:], v32[:], ident[:], is_transpose=True
                    )
                    vT = sqp.tile([half, JM], F32, tag="vTs", bufs=1)
                    nc.scalar.activation(vT[:], vT_ps[:], AF.Copy)
                    nc.sync.dma_start(
                        v_out[:].rearrange("b j m -> b (j m)")[
                            b0 : b0 + half, :
                        ],
                        vT[:],
                    )


_NC_CACHE = {}


def _build_nc(b_c=B_C, half=16, b_blk=2, reps=1, stage=3, split_waits=True):
    key = (b_c, half, b_blk, reps, stage, split_waits)
    if key not in _NC_CACHE:
        _patch_tile()
        nc = bass.Bass("TRN2", target_bir_lowering=False, debug=False)
        wcr_d = nc.dram_tensor("wcr", [128, H, JM], FP8, kind="ExternalInput").ap()
        xblk_d = nc.dram_tensor(
            "xblk", [b_c // half, T, 128, CH_T, half, IP], FP8,
            kind="ExternalInput",
        ).ap()
        xt_d = nc.dram_tensor("xt", [128, H, b_c], FP8, kind="ExternalInput").ap()
        mask_d = nc.dram_tensor("mask", [JM, J], BF16, kind="ExternalInput").ap()
        ident_d = nc.dram_tensor("ident", [128, 128], F32, kind="ExternalInput").ap()
        v_d = nc.dram_tensor("v", [b_c, J, M], F32, kind="ExternalOutput").ap()
        with tile.TileContext(nc) as tc:
            build_kernel(
                tc,
                [v_d],
                [wcr_d, xblk_d, xt_d, mask_d, ident_d],
                b_c=b_c,
                half=half,
                b_blk=b_blk,
                reps=reps,
                stage=stage,
            )
        if split_waits:
            _split_waits(nc)
        _NC_CACHE[key] = nc
    return _NC_CACHE[key]


def host_prep(x, W):
    """Returns (wcr, xblk_all, xt_all, mask, ident); x-deriveds cover all B.
    Row order of the 128 K-rows is (i16, n): i = h*IP + i16."""
    bf = ml_dtypes.bfloat16
    f8 = ml_dtypes.bfloat16
    nb = x.shape[0]
    # wcr[(i16*N + n), h, jm] = W[h*IP + i16, j, n, m]
    Wr = np.ascontiguousarray(W.transpose(0, 2, 1, 3)).reshape(I, N, JM)
    Wr = Wr.reshape(H, IP, N, JM)
    wcr = np.ascontiguousarray(Wr.transpose(1, 2, 0, 3)).reshape(128, H, JM)
    # x rows in the same (i16, n) order per h
    xr = x.reshape(nb, H, IP, N)
    xrows = np.ascontiguousarray(xr.transpose(2, 3, 1, 0)).reshape(128, H, nb)
    rows = np.arange(128)
    i16_of_row = rows // N
    xblk = np.zeros((128, H, nb, IP), np.float32)
    for r in range(128):
        xblk[r, :, :, i16_of_row[r]] = xrows[r]
    mask = np.zeros((JM, J), np.float32)
    for j in range(J):
        mask[j * M : (j + 1) * M, j] = 1.0
    ident = np.eye(128, dtype=np.float32)
    return (
        wcr.astype(f8),
        xblk.astype(f8),
        xrows.astype(f8),
        mask.astype(bf),
        ident,
    )


def regroup(xblk_core, xt_core, half, b_blk=None):
    """xblk [128,H,nb,IP] -> t-major [n_half, T, 128, CH_T, half, IP];
    xt passes through."""
    nb = xblk_core.shape[2]
    n_half = nb // half
    xb = xblk_core.reshape(128, T, CH_T, nb, IP)
    xb = xb.transpose(3, 1, 0, 2, 4)  # [nb, T, 128, CH_T, IP]
    xb = xb.reshape(n_half, half, T, 128, CH_T, IP).transpose(0, 2, 3, 4, 1, 5)
    return np.ascontiguousarray(xb), np.ascontiguousarray(xt_core)


def kernel(x, W):
    x = np.asarray(x, np.float32)
    W = np.asarray(W, np.float32)
    wcr, xblk_all, xt_all, mask, ident = host_prep(x, W)
    nc = _build_nc()
    in_maps = []
    for c in range(N_CORES):
        bs = slice(c * B_C, (c + 1) * B_C)
        xb_c, xt_c = regroup(xblk_all[:, :, bs, :], xt_all[:, :, bs], 16, 2)
        in_maps.append(
            {"wcr": wcr, "xblk": xb_c, "xt": xt_c, "mask": mask, "ident": ident}
        )
    res = run_bass_kernel_spmd(nc, in_maps, list(range(N_CORES)))
    out = np.concatenate([res.results[c]["v"] for c in range(N_CORES)], axis=0)
    return out.astype(np.float32)

